# revision 6
# baseline (speedup 1.0000x reference)
"""AttnDecoder step on 8 TRN2 NeuronCores (Bass/Tile).

Strategy (vocab tensor-parallel):
  - Host gathers the embedding row (emb[token]) so the 524MB table never
    touches the device.
  - out_W/out_b sharded over vocab: 16000 rows per core. Host pre-transposes
    and row-interleaves so every device DMA is contiguous.
  - Attention (tiny) replicated on all cores; comb_W / GRU weights sharded
    8-way over their output dims with AllGathers to reassemble the [1,1024]
    activations.
  - log-softmax via local sum(exp(logits)) + one tiny AllGather (no max pass
    needed: logits are O(1) so exp is f32-safe).
  - All biases folded into the matmuls via an augmented all-but-one-zero
    contraction chunk (except out_b which is added in the epilogue).

Device layout conventions (host compensates by permuting weight rows):
  - "chunk" layout  : col k of a [128,K] tile holds v[k*128+p]   (p=partition)
  - "ilv8" layout   : col k of a [128,8] tile holds v[p*8+k]  -> contiguous
    32B-per-partition DMA from a flat [1024] DRAM buffer.
"""

import os
import sys
import numpy as np

for _p in ("/opt/trn_rl_repo",):
    if _p not in sys.path:
        sys.path.append(_p)

H = 1024
V = 128000
L = 100
NCORES = 8
VS = V // NCORES          # 16000 vocab rows per core
NSLICE = 500              # logits per PSUM slice (one 2KB bank of f32)
TGC = 2000                # vocab cols per streamed weight tile (1MB DMA)
NTG = VS // TGC           # 8 tile groups
NSPT = TGC // NSLICE      # 4 slices per tile group
ROWS = 128
JW = VS // ROWS           # 125 logits per partition in the square layout

LAST_EXEC_NS = None
LAST_RESULTS = None
_BUILD_CACHE = {}

_AXON_SO = "/opt/axon/libaxon_pjrt.so"


def _install_profile_shims():
    """Provide the missing ``antenv.axon_hooks`` registry + NTFF hook so
    run_bass_kernel_spmd(trace=True) works on this image. Profiling only —
    the plain execution path never touches this."""
    import types
    import ctypes
    import contextlib

    if "antenv.axon_hooks" not in sys.modules:
        mod = types.ModuleType("antenv.axon_hooks")
        _hook = [None]
        mod.set_axon_ntff_profile_hook = lambda h: _hook.__setitem__(0, h)
        mod.get_axon_ntff_profile_hook = lambda: _hook[0]
        sys.modules["antenv.axon_hooks"] = mod

    from antenv.axon_hooks import (  # type: ignore
        get_axon_ntff_profile_hook,
        set_axon_ntff_profile_hook,
    )

    if get_axon_ntff_profile_hook() is None and os.path.exists(_AXON_SO):
        lib = ctypes.CDLL(_AXON_SO)
        if hasattr(lib, "axon_start_nrt_profile"):
            lib.axon_start_nrt_profile.argtypes = [
                ctypes.POINTER(ctypes.c_int64),
                ctypes.c_size_t,
            ]
            lib.axon_start_nrt_profile.restype = ctypes.c_int64
            lib.axon_stop_nrt_profile.argtypes = [ctypes.c_char_p]
            lib.axon_stop_nrt_profile.restype = ctypes.c_int64

            @contextlib.contextmanager
            def _hook_cm(output_dir, device_ids):
                import jax
                jax.devices()
                if device_ids:
                    ids = (ctypes.c_int64 * len(device_ids))(*device_ids)
                    rc = lib.axon_start_nrt_profile(ids, len(device_ids))
                else:
                    rc = lib.axon_start_nrt_profile(None, 0)
                if rc != 0:
                    raise RuntimeError(f"axon_start_nrt_profile rc={rc}")
                try:
                    yield
                finally:
                    n = lib.axon_stop_nrt_profile(str(output_dir).encode())
                    print(f"profile: {n} ntff file(s) -> {output_dir}",
                          file=sys.stderr)

            set_axon_ntff_profile_hook(_hook_cm)

    # keep artifacts local; no bucket in this container
    from concourse import bass_utils
    bass_utils.upload_artifacts = lambda tmpdir: f"local:{tmpdir}"


def _build_nc():
    import concourse.bass as bass
    import concourse.bacc as bacc
    import concourse.tile as tile
    from concourse import mybir

    f32 = mybir.dt.float32
    AF = mybir.ActivationFunctionType

    nc = bacc.Bacc(num_devices=NCORES)

    # ---------------- parameters (per-core data, identical graph) ----------
    p_eh = nc.declare_dram_parameter("eh_cols", [128, 17], f32, isOutput=False)
    p_attnwt = nc.declare_dram_parameter("attnwt", [128, 17 * L], f32, isOutput=False)
    p_enc = nc.declare_dram_parameter("enc", [L, H], f32, isOutput=False)
    p_emb = nc.declare_dram_parameter("emb_cols", [128, 8], f32, isOutput=False)
    p_combwt = nc.declare_dram_parameter("combwt", [128, 17 * 128], f32, isOutput=False)
    p_xw = nc.declare_dram_parameter("xw", [128, 9 * 384], f32, isOutput=False)
    p_hw = nc.declare_dram_parameter("hw", [128, 9 * 384], f32, isOutput=False)
    p_hcols = nc.declare_dram_parameter("h_cols", [128, 9], f32, isOutput=False)
    p_hchunk = nc.declare_dram_parameter("h_chunk", [1, 128], f32, isOutput=False)
    p_consts = nc.declare_dram_parameter("consts", [128, 2], f32, isOutput=False)
    p_onesrow = nc.declare_dram_parameter("ones_row", [1, 128], f32, isOutput=False)
    p_outb = nc.declare_dram_parameter("outb_sq", [ROWS, JW], f32, isOutput=False)
    p_outwt = nc.declare_dram_parameter("outwt", [8, 128, VS], f32, isOutput=False)

    o_logits = nc.declare_dram_parameter("out_logits", [VS], f32, isOutput=True)
    o_h = nc.declare_dram_parameter("out_h", [H], f32, isOutput=True)
    o_attnw = nc.declare_dram_parameter("out_attnw", [L], f32, isOutput=True)

    RG = [list(range(NCORES))]

    with tile.TileContext(nc) as tc:
        with (
            tc.tile_pool(name="sb", bufs=1) as sb,
            tc.tile_pool(name="big", bufs=12) as bigp,
            tc.tile_pool(name="sl", bufs=4) as slp,
            tc.tile_pool(name="ps", bufs=4, space="PSUM") as psp,
            tc.tile_pool(name="psbig", bufs=4, space="PSUM") as psbig,
            tc.tile_pool(name="dram", bufs=1, space="DRAM") as dram,
        ):
            # ---------- persistent SBUF loads (latency-critical first) -----
            t_eh = sb.tile([128, 17], f32, tag="t_eh")
            t_attnwt = sb.tile([128, 17 * L], f32, tag="t_attnwt")
            t_enc = sb.tile([L, H], f32, tag="t_enc")
            t_emb = sb.tile([128, 8], f32, tag="t_emb")
            t_consts = sb.tile([128, 2], f32, tag="t_consts")
            t_ones = sb.tile([1, 128], f32, tag="t_ones")
            t_combwt = sb.tile([128, 17 * 128], f32, tag="t_combwt")
            t_xw = sb.tile([128, 9 * 384], f32, tag="t_xw")
            t_hw = sb.tile([128, 9 * 384], f32, tag="t_hw")
            t_hcols = sb.tile([128, 9], f32, tag="t_hcols")
            t_hchunk = sb.tile([1, 128], f32, tag="t_hchunk")
            t_outb = sb.tile([ROWS, JW], f32, tag="t_outb")

            nc.sync.dma_start(t_eh[:], p_eh[:])
            nc.sync.dma_start(t_attnwt[:], p_attnwt[:])
            nc.sync.dma_start(t_consts[:], p_consts[:])
            nc.sync.dma_start(t_ones[:], p_onesrow[:])
            nc.sync.dma_start(t_enc[:], p_enc[:])
            nc.sync.dma_start(t_emb[:], p_emb[:])
            nc.sync.dma_start(t_combwt[:], p_combwt[:])
            nc.sync.dma_start(t_xw[:], p_xw[:])
            nc.sync.dma_start(t_hw[:], p_hw[:])
            nc.sync.dma_start(t_hcols[:], p_hcols[:])
            nc.sync.dma_start(t_hchunk[:], p_hchunk[:])
            nc.sync.dma_start(t_outb[:], p_outb[:])

            # ---------- ACT table warmup (off critical path) ---------------
            warm = sb.tile([1, 4], f32, tag="warm")
            nc.vector.memset(warm[:], 0.0)
            for fn in (AF.Exp, AF.Sigmoid, AF.Tanh, AF.Ln, AF.Relu):
                nc.scalar.activation(warm[:, 0:1], warm[:, 0:1], fn)

            # ---------- attention ------------------------------------------
            # attn_logits[100,1] = sum_k attnwt_k.T @ eh_k   (bias chunk k=16)
            ps_al = psp.tile([100, 1], f32, tag="ps")
            for k in range(17):
                nc.tensor.matmul(
                    ps_al[:],
                    t_attnwt[:, k * L:(k + 1) * L],
                    t_eh[:, k:k + 1],
                    start=(k == 0),
                    stop=(k == 16),
                )
            expw = sb.tile([100, 1], f32, tag="expw")
            nc.scalar.activation(expw[:], ps_al[:], AF.Exp)
            # S = sum_p exp ; via ones matmul
            ps_s = psp.tile([1, 1], f32, tag="ps")
            nc.tensor.matmul(ps_s[:], expw[:], t_consts[0:100, 1:2], start=True, stop=True)
            rs = sb.tile([1, 1], f32, tag="rs")
            nc.vector.reciprocal(rs[:], ps_s[:])
            # broadcast 1/S across partitions: ones_row.T @ rs
            ps_rb = psp.tile([128, 1], f32, tag="ps")
            nc.tensor.matmul(ps_rb[:], t_ones[:], rs[:], start=True, stop=True)
            rb = sb.tile([128, 1], f32, tag="rb")
            nc.vector.tensor_copy(rb[:], ps_rb[:])
            wn = sb.tile([100, 1], f32, tag="wn")
            nc.vector.tensor_mul(wn[:], expw[:], rb[0:100, :])
            nc.sync.dma_start(o_attnw[:], wn[:])

            # attn_applied.T in chunk layout: [128, 8]
            cat = sb.tile([128, 17], f32, tag="cat")
            nc.sync.dma_start(cat[:, 0:8], p_emb[:])
            nc.vector.tensor_copy(cat[:, 16:17], t_consts[:, 0:1])
            ps_at = psp.tile([128, 8], f32, tag="ps")
            for m in range(8):
                nc.tensor.matmul(
                    ps_at[:, m:m + 1],
                    t_enc[:, m * 128:(m + 1) * 128],
                    wn[:],
                    start=True,
                    stop=True,
                )
            nc.vector.tensor_copy(cat[:, 8:16], ps_at[:])

            # ---------- combine + relu (sharded over output dim) -----------
            ps_x = psp.tile([1, 128], f32, tag="ps")
            for k in range(17):
                nc.tensor.matmul(
                    ps_x[:],
                    cat[:, k:k + 1],
                    t_combwt[:, k * 128:(k + 1) * 128],
                    start=(k == 0),
                    stop=(k == 16),
                )
            x_sb = sb.tile([1, 128], f32, tag="x_sb")
            nc.scalar.activation(x_sb[:], ps_x[:], AF.Relu)

            d_xin = dram.tile([128], f32, tag="d_xin")
            d_xag = dram.tile([H], f32, tag="d_xag")
            nc.sync.dma_start(d_xin[:], x_sb[:])
            nc.gpsimd.collective_compute(
                "AllGather",
                mybir.AluOpType.bypass,
                replica_groups=RG,
                ins=[d_xin[:].opt()],
                outs=[d_xag[:].opt()],
            )
            x_cols = sb.tile([128, 9], f32, tag="x_cols")
            nc.sync.dma_start(
                x_cols[:, 0:8], d_xag[:].rearrange("(p k) -> p k", k=8)
            )
            nc.vector.tensor_copy(x_cols[:, 8:9], t_consts[:, 0:1])

            # ---------- GRU (gates sharded: this core's 128 rows each) -----
            ps_grz = psp.tile([1, 256], f32, tag="ps")
            ps_gin = psp.tile([1, 128], f32, tag="ps")
            ps_ghn = psp.tile([1, 128], f32, tag="ps")
            # r/z: x-side and h-side accumulate into ONE psum group
            for k in range(9):
                nc.tensor.matmul(
                    ps_grz[:], x_cols[:, k:k + 1],
                    t_xw[:, k * 384:k * 384 + 256],
                    start=(k == 0), stop=False,
                )
            for k in range(9):
                nc.tensor.matmul(
                    ps_grz[:], t_hcols[:, k:k + 1],
                    t_hw[:, k * 384:k * 384 + 256],
                    start=False, stop=(k == 8),
                )
            for k in range(9):
                nc.tensor.matmul(
                    ps_gin[:], x_cols[:, k:k + 1],
                    t_xw[:, k * 384 + 256:(k + 1) * 384],
                    start=(k == 0), stop=(k == 8),
                )
            for k in range(9):
                nc.tensor.matmul(
                    ps_ghn[:], t_hcols[:, k:k + 1],
                    t_hw[:, k * 384 + 256:(k + 1) * 384],
                    start=(k == 0), stop=(k == 8),
                )
            rz = sb.tile([1, 256], f32, tag="rz")
            nc.scalar.activation(rz[:], ps_grz[:], AF.Sigmoid)
            t2 = sb.tile([1, 128], f32, tag="t2")
            nc.vector.tensor_mul(t2[:], rz[:, 0:128], ps_ghn[:])
            t3 = sb.tile([1, 128], f32, tag="t3")
            nc.vector.tensor_add(t3[:], ps_gin[:], t2[:])
            n_t = sb.tile([1, 128], f32, tag="n_t")
            nc.scalar.activation(n_t[:], t3[:], AF.Tanh)
            t4 = sb.tile([1, 128], f32, tag="t4")
            nc.vector.tensor_sub(t4[:], t_hchunk[:], n_t[:])
            t5 = sb.tile([1, 128], f32, tag="t5")
            nc.vector.tensor_mul(t5[:], rz[:, 128:256], t4[:])
            hn = sb.tile([1, 128], f32, tag="hn")
            nc.vector.tensor_add(hn[:], n_t[:], t5[:])

            d_hin = dram.tile([128], f32, tag="d_hin")
            d_hag = dram.tile([H], f32, tag="d_hag")
            nc.sync.dma_start(d_hin[:], hn[:])
            nc.gpsimd.collective_compute(
                "AllGather",
                mybir.AluOpType.bypass,
                replica_groups=RG,
                ins=[d_hin[:].opt()],
                outs=[d_hag[:].opt()],
            )
            nc.sync.dma_start(o_h[:], d_hag[:])
            h2 = sb.tile([128, 8], f32, tag="h2")
            nc.sync.dma_start(h2[:], d_hag[:].rearrange("(p k) -> p k", k=8))

            # ---------- big vocab matmul (streamed) ------------------------
            d_logits = dram.tile([VS], f32, tag="d_logits")
            for tg in range(NTG):
                tiles = []
                for k in range(8):
                    tw = bigp.tile([128, TGC], f32, tag="bw")
                    nc.gpsimd.dma_start(
                        tw[:], p_outwt[k, :, tg * TGC:(tg + 1) * TGC]
                    )
                    tiles.append(tw)
                for n in range(NSPT):
                    ps_b = psbig.tile([1, NSLICE], f32, tag="bps")
                    for k in range(8):
                        nc.tensor.matmul(
                            ps_b[:],
                            h2[:, k:k + 1],
                            tiles[k][:, n * NSLICE:(n + 1) * NSLICE],
                            start=(k == 0),
                            stop=(k == 7),
                        )
                    sl = slp.tile([1, NSLICE], f32, tag="sl")
                    nc.vector.tensor_copy(sl[:], ps_b[:])
                    off = (tg * NSPT + n) * NSLICE
                    nc.sync.dma_start(d_logits[off:off + NSLICE], sl[:])

            # ---------- epilogue: log-softmax over the full vocab ----------
            lsq = sb.tile([ROWS, JW], f32, tag="lsq")
            nc.sync.dma_start(lsq[:], d_logits[:].rearrange("(p j) -> p j", j=JW))
            nc.vector.tensor_add(lsq[:], lsq[:], t_outb[:])
            esq = sb.tile([ROWS, JW], f32, tag="esq")
            rowsum = sb.tile([ROWS, 1], f32, tag="rowsum")
            nc.scalar.activation(esq[:], lsq[:], AF.Exp, accum_out=rowsum[:])
            ps_t = psp.tile([1, 1], f32, tag="ps")
            nc.tensor.matmul(ps_t[:], rowsum[:], t_consts[:, 1:2], start=True, stop=True)
            s_st = sb.tile([1, 8], f32, tag="s_st")
            nc.vector.memset(s_st[:], 0.0)
            nc.vector.tensor_copy(s_st[:, 0:1], ps_t[:])
            d_sin = dram.tile([8], f32, tag="d_sin")
            d_sag = dram.tile([8 * NCORES], f32, tag="d_sag")
            nc.sync.dma_start(d_sin[:], s_st[:])
            nc.gpsimd.collective_compute(
                "AllGather",
                mybir.AluOpType.bypass,
                replica_groups=RG,
                ins=[d_sin[:].opt()],
                outs=[d_sag[:].opt()],
            )
            s_row = sb.tile([1, 8 * NCORES], f32, tag="s_row")
            nc.sync.dma_start(s_row[:], d_sag[:])
            stot = sb.tile([1, 1], f32, tag="stot")
            nc.vector.reduce_sum(stot[:], s_row[:], axis=mybir.AxisListType.X)
            lse = sb.tile([1, 1], f32, tag="lse")
            nc.scalar.activation(lse[:], stot[:], AF.Ln)
            ps_lb = psp.tile([128, 1], f32, tag="ps")
            nc.tensor.matmul(ps_lb[:], t_ones[:], lse[:], start=True, stop=True)
            lseb = sb.tile([128, 1], f32, tag="lseb")
            nc.vector.tensor_copy(lseb[:], ps_lb[:])
            fin = sb.tile([ROWS, JW], f32, tag="fin")
            nc.vector.tensor_scalar(
                fin[:], lsq[:], lseb[:], None,
                op0=mybir.AluOpType.subtract,
            )
            nc.sync.dma_start(
                o_logits[:].rearrange("(p j) -> p j", j=JW), fin[:]
            )

    return nc


def _host_prep(inputs):
    token = int(np.asarray(inputs["token"]).reshape(-1)[0])
    emb = np.asarray(inputs["emb"], dtype=np.float32)
    embedded = emb[token].astype(np.float32)                    # [H]
    h = np.asarray(inputs["hidden"], dtype=np.float32).reshape(H)
    enc = np.ascontiguousarray(np.asarray(inputs["encoder_outputs"], np.float32))
    attn_W = np.asarray(inputs["attn_W"], np.float32)           # [L, 2H]
    attn_b = np.asarray(inputs["attn_b"], np.float32)           # [L]
    comb_W = np.asarray(inputs["comb_W"], np.float32)           # [H, 2H]
    comb_b = np.asarray(inputs["comb_b"], np.float32)           # [H]
    W_ih = np.asarray(inputs["W_ih"], np.float32)               # [3H, H]
    W_hh = np.asarray(inputs["W_hh"], np.float32)
    b_ih = np.asarray(inputs["b_ih"], np.float32)
    b_hh = np.asarray(inputs["b_hh"], np.float32)
    out_W = np.asarray(inputs["out_W"], np.float32)             # [V, H]
    out_b = np.asarray(inputs["out_b"], np.float32)             # [V]

    eh = np.concatenate([embedded, h])                          # [2H]
    eh_cols = np.zeros((128, 17), np.float32)
    eh_cols[:, :16] = eh.reshape(16, 128).T
    eh_cols[0, 16] = 1.0

    attnwt = np.zeros((128, 17 * L), np.float32)
    attnwt[:, :16 * L] = (
        attn_W.T.reshape(16, 128, L).transpose(1, 0, 2).reshape(128, 16 * L)
    )
    attnwt[0, 16 * L:] = attn_b

    emb_cols = np.ascontiguousarray(embedded.reshape(8, 128).T)

    h_cols = np.zeros((128, 9), np.float32)
    h_cols[:, :8] = h.reshape(128, 8)
    h_cols[0, 8] = 1.0

    consts = np.zeros((128, 2), np.float32)
    consts[0, 0] = 1.0
    consts[:, 1] = 1.0
    ones_row = np.ones((1, 128), np.float32)

    common = {
        "eh_cols": eh_cols, "attnwt": attnwt, "enc": enc,
        "emb_cols": emb_cols, "h_cols": h_cols,
        "consts": consts, "ones_row": ones_row,
    }

    in_maps = []
    for c in range(NCORES):
        m = dict(common)
        blk = comb_W[c * 128:(c + 1) * 128, :]                  # [128, 2H]
        combwt = np.zeros((128, 17 * 128), np.float32)
        combwt[:, :16 * 128] = (
            blk.T.reshape(16, 128, 128).transpose(1, 0, 2).reshape(128, 16 * 128)
        )
        combwt[0, 16 * 128:] = comb_b[c * 128:(c + 1) * 128]
        m["combwt"] = combwt

        gidx = np.concatenate([
            np.arange(128) + c * 128,
            np.arange(128) + H + c * 128,
            np.arange(128) + 2 * H + c * 128,
        ])
        for nm, W, b in (("xw", W_ih, b_ih), ("hw", W_hh, b_hh)):
            Wg = W[gidx, :]                                     # [384, H]
            t = np.zeros((128, 9 * 384), np.float32)
            t[:, :8 * 384] = Wg.T.reshape(128, 8, 384).reshape(128, 8 * 384)
            t[0, 8 * 384:] = b[gidx]
            m[nm] = t

        m["h_chunk"] = h[c * 128:(c + 1) * 128][None, :].copy()

        shard = out_W[c * VS:(c + 1) * VS, :]                   # [VS, H]
        m["outwt"] = np.ascontiguousarray(
            shard.T.reshape(128, 8, VS).transpose(1, 0, 2)
        )
        m["outb_sq"] = np.ascontiguousarray(
            out_b[c * VS:(c + 1) * VS].reshape(ROWS, JW)
        )
        in_maps.append(m)
    return in_maps


def kernel(**inputs):
    global LAST_EXEC_NS, LAST_RESULTS
    from concourse.bass_utils import run_bass_kernel_spmd

    if "nc" not in _BUILD_CACHE:
        nc = _build_nc()
        if not nc.is_finalized():
            nc.finalize()
        _BUILD_CACHE["nc"] = nc
    nc = _BUILD_CACHE["nc"]

    in_maps = _host_prep(inputs)
    trace = os.environ.get("KERNEL_PROFILE", "0") == "1"
    kw = {}
    if trace:
        try:
            _install_profile_shims()
            kw["tmpdir"] = os.environ.get("KERNEL_TRACE_DIR") or None
        except Exception as e:  # profiling is best-effort
            print(f"profile shim failed: {e}", file=sys.stderr)
            trace = False
    res = run_bass_kernel_spmd(
        nc, in_maps, core_ids=list(range(NCORES)), trace=trace, **kw,
    )
    LAST_EXEC_NS = getattr(res, "exec_time_ns", None)
    LAST_RESULTS = res

    outs = res.results
    logits = np.concatenate(
        [np.asarray(outs[c]["out_logits"], np.float32) for c in range(NCORES)]
    )[None, :]
    h_new = np.asarray(outs[0]["out_h"], np.float32)[None, None, :]
    attnw = np.asarray(outs[0]["out_attnw"], np.float32)[None, :]
    return logits, h_new, attnw


# revision 16
# speedup vs baseline: 1.5346x; 1.5346x over previous
"""AttnDecoder step on 8 TRN2 NeuronCores (Bass/Tile).

Strategy (vocab tensor-parallel):
  - Host gathers the embedding row (emb[token]) so the 524MB table never
    touches the device.
  - out_W/out_b sharded over vocab: 16000 rows per core. Host pre-transposes
    and row-interleaves so every device DMA is contiguous.
  - Attention (tiny) replicated on all cores; comb_W / GRU weights sharded
    8-way over their output dims with AllGathers to reassemble the [1,1024]
    activations.
  - log-softmax via local sum(exp(logits)) + one tiny AllGather (no max pass
    needed: logits are O(1) so exp is f32-safe).
  - All biases folded into the matmuls via an augmented all-but-one-zero
    contraction chunk (except out_b which is added in the epilogue).

Device layout conventions (host compensates by permuting weight rows):
  - "chunk" layout  : col k of a [128,K] tile holds v[k*128+p]   (p=partition)
  - "ilv8" layout   : col k of a [128,8] tile holds v[p*8+k]  -> contiguous
    32B-per-partition DMA from a flat [1024] DRAM buffer.
"""

import os
import sys
import numpy as np

for _p in ("/opt/trn_rl_repo",):
    if _p not in sys.path:
        sys.path.append(_p)

H = 1024
V = 128000
L = 100
NCORES = 8
VS = V // NCORES          # 16000 vocab rows per core
NSLICE = 500              # logits per PSUM slice (one 2KB bank of f32)
TGC = 2000                # vocab cols per streamed weight tile (0.5MB bf16 DMA)
NTG = VS // TGC           # 8 tile groups
NSPT = TGC // NSLICE      # 4 slices per tile group
NSL = VS // NSLICE        # 32 slices total
ROWS = 128
JW = VS // ROWS           # 125 logits per partition in the square layout

LAST_EXEC_NS = None
LAST_RESULTS = None
_BUILD_CACHE = {}

_AXON_SO = "/opt/axon/libaxon_pjrt.so"


def _install_profile_shims():
    """Provide the missing ``antenv.axon_hooks`` registry + NTFF hook so
    run_bass_kernel_spmd(trace=True) works on this image. Profiling only —
    the plain execution path never touches this."""
    import types
    import ctypes
    import contextlib

    if "antenv.axon_hooks" not in sys.modules:
        mod = types.ModuleType("antenv.axon_hooks")
        _hook = [None]
        mod.set_axon_ntff_profile_hook = lambda h: _hook.__setitem__(0, h)
        mod.get_axon_ntff_profile_hook = lambda: _hook[0]
        sys.modules["antenv.axon_hooks"] = mod

    from antenv.axon_hooks import (  # type: ignore
        get_axon_ntff_profile_hook,
        set_axon_ntff_profile_hook,
    )

    if get_axon_ntff_profile_hook() is None and os.path.exists(_AXON_SO):
        lib = ctypes.CDLL(_AXON_SO)
        if hasattr(lib, "axon_start_nrt_profile"):
            lib.axon_start_nrt_profile.argtypes = [
                ctypes.POINTER(ctypes.c_int64),
                ctypes.c_size_t,
            ]
            lib.axon_start_nrt_profile.restype = ctypes.c_int64
            lib.axon_stop_nrt_profile.argtypes = [ctypes.c_char_p]
            lib.axon_stop_nrt_profile.restype = ctypes.c_int64

            @contextlib.contextmanager
            def _hook_cm(output_dir, device_ids):
                import jax
                jax.devices()
                if device_ids:
                    ids = (ctypes.c_int64 * len(device_ids))(*device_ids)
                    rc = lib.axon_start_nrt_profile(ids, len(device_ids))
                else:
                    rc = lib.axon_start_nrt_profile(None, 0)
                if rc != 0:
                    raise RuntimeError(f"axon_start_nrt_profile rc={rc}")
                try:
                    yield
                finally:
                    n = lib.axon_stop_nrt_profile(str(output_dir).encode())
                    print(f"profile: {n} ntff file(s) -> {output_dir}",
                          file=sys.stderr)

            set_axon_ntff_profile_hook(_hook_cm)

    # keep artifacts local; no bucket in this container
    from concourse import bass_utils
    bass_utils.upload_artifacts = lambda tmpdir: f"local:{tmpdir}"


def _build_nc():
    import concourse.bass as bass
    import concourse.bacc as bacc
    import concourse.tile as tile
    from concourse import mybir

    f32 = mybir.dt.float32
    bf16 = mybir.dt.bfloat16
    AF = mybir.ActivationFunctionType

    nc = bacc.Bacc(num_devices=NCORES)

    # ---------------- parameters (per-core data, identical graph) ----------
    p_eh = nc.declare_dram_parameter("eh_cols", [128, 17], f32, isOutput=False)
    p_attnwt = nc.declare_dram_parameter("attnwt", [128, 17 * L], f32, isOutput=False)
    p_enc = nc.declare_dram_parameter("enc", [L, H], f32, isOutput=False)
    p_emb = nc.declare_dram_parameter("emb_cols", [128, 8], f32, isOutput=False)
    p_combwt = nc.declare_dram_parameter("combwt", [128, 17 * 128], f32, isOutput=False)
    p_xw = nc.declare_dram_parameter("xw", [128, 9 * 384], f32, isOutput=False)
    p_hw = nc.declare_dram_parameter("hw", [128, 9 * 384], f32, isOutput=False)
    p_hcols = nc.declare_dram_parameter("h_cols", [128, 9], f32, isOutput=False)
    p_hchunk = nc.declare_dram_parameter("h_chunk", [1, 128], f32, isOutput=False)
    p_consts = nc.declare_dram_parameter("consts", [128, 2], f32, isOutput=False)
    p_onesrow = nc.declare_dram_parameter("ones_row", [1, 128], f32, isOutput=False)
    p_outb = nc.declare_dram_parameter("outb_p0", [1, VS], f32, isOutput=False)
    p_outwt = nc.declare_dram_parameter("outwt", [8, 128, VS], bf16, isOutput=False)

    o_logits = nc.declare_dram_parameter("out_logits", [VS], f32, isOutput=True)
    o_h = nc.declare_dram_parameter("out_h", [H], f32, isOutput=True)
    o_attnw = nc.declare_dram_parameter("out_attnw", [L], f32, isOutput=True)

    RG = [list(range(NCORES))]

    with tile.TileContext(nc) as tc:
        with (
            tc.tile_pool(name="sb", bufs=1) as sb,
            tc.tile_pool(name="big", bufs=16) as bigp,
            tc.tile_pool(name="sl", bufs=4) as slp,
            tc.tile_pool(name="ps", bufs=4, space="PSUM") as psp,
            tc.tile_pool(name="psbig", bufs=4, space="PSUM") as psbig,
            tc.tile_pool(name="dram", bufs=1, space="DRAM") as dram,
        ):
            # ---------- persistent SBUF loads (latency-critical first) -----
            t_eh = sb.tile([128, 17], f32, tag="t_eh")
            t_attnwt = sb.tile([128, 17 * L], f32, tag="t_attnwt")
            t_enc = sb.tile([L, H], f32, tag="t_enc")
            t_emb = sb.tile([128, 8], f32, tag="t_emb")
            t_consts = sb.tile([128, 2], f32, tag="t_consts")
            t_ones = sb.tile([1, 128], f32, tag="t_ones")
            t_combwt = sb.tile([128, 17 * 128], f32, tag="t_combwt")
            t_xw = sb.tile([128, 9 * 384], f32, tag="t_xw")
            t_hw = sb.tile([128, 9 * 384], f32, tag="t_hw")
            t_hcols = sb.tile([128, 9], f32, tag="t_hcols")
            t_hchunk = sb.tile([1, 128], f32, tag="t_hchunk")
            t_outb = sb.tile([1, VS], f32, tag="t_outb")

            # warm up the collective path so later AGs are cheap
            d_win = dram.tile([8], f32, tag="d_win")
            d_wout = dram.tile([8 * NCORES], f32, tag="d_wout")
            nc.gpsimd.collective_compute(
                "AllGather",
                mybir.AluOpType.bypass,
                replica_groups=RG,
                ins=[d_win[:].opt()],
                outs=[d_wout[:].opt()],
            )

            nc.sync.dma_start(t_eh[:], p_eh[:])
            nc.sync.dma_start(t_attnwt[:], p_attnwt[:])
            nc.sync.dma_start(t_consts[:], p_consts[:])
            nc.sync.dma_start(t_ones[:], p_onesrow[:])
            nc.sync.dma_start(t_enc[:], p_enc[:])
            nc.sync.dma_start(t_emb[:], p_emb[:])
            nc.sync.dma_start(t_combwt[:], p_combwt[:])
            nc.sync.dma_start(t_xw[:], p_xw[:])
            nc.sync.dma_start(t_hw[:], p_hw[:])
            nc.sync.dma_start(t_hcols[:], p_hcols[:])
            nc.sync.dma_start(t_hchunk[:], p_hchunk[:])
            nc.sync.dma_start(t_outb[:], p_outb[:])

            # ---------- ACT table warmup (off critical path) ---------------
            warm = sb.tile([1, 4], f32, tag="warm")
            nc.vector.memset(warm[:], 0.0)
            for fn in (AF.Exp, AF.Sigmoid, AF.Tanh, AF.Ln, AF.Relu):
                nc.scalar.activation(warm[:, 0:1], warm[:, 0:1], fn)

            # ---------- attention ------------------------------------------
            # attn_logits[100,1] = sum_k attnwt_k.T @ eh_k   (bias chunk k=16)
            ps_al = psp.tile([100, 1], f32, tag="ps")
            for k in range(17):
                nc.tensor.matmul(
                    ps_al[:],
                    t_attnwt[:, k * L:(k + 1) * L],
                    t_eh[:, k:k + 1],
                    start=(k == 0),
                    stop=(k == 16),
                )
            expw = sb.tile([100, 1], f32, tag="expw")
            nc.scalar.activation(expw[:], ps_al[:], AF.Exp)
            # S = sum_p exp ; via ones matmul
            ps_s = psp.tile([1, 1], f32, tag="ps")
            nc.tensor.matmul(ps_s[:], expw[:], t_consts[0:100, 1:2], start=True, stop=True)
            rs = sb.tile([1, 1], f32, tag="rs")
            nc.vector.reciprocal(rs[:], ps_s[:])
            # broadcast 1/S across partitions: ones_row.T @ rs
            ps_rb = psp.tile([128, 1], f32, tag="ps")
            nc.tensor.matmul(ps_rb[:], t_ones[:], rs[:], start=True, stop=True)
            rb = sb.tile([128, 1], f32, tag="rb")
            nc.vector.tensor_copy(rb[:], ps_rb[:])
            wn = sb.tile([100, 1], f32, tag="wn")
            nc.vector.tensor_mul(wn[:], expw[:], rb[0:100, :])
            nc.sync.dma_start(o_attnw[:], wn[:])

            # attn_applied.T in chunk layout: [128, 8]
            cat = sb.tile([128, 17], f32, tag="cat")
            nc.sync.dma_start(cat[:, 0:8], p_emb[:])
            nc.vector.tensor_copy(cat[:, 16:17], t_consts[:, 0:1])
            ps_at = psp.tile([128, 8], f32, tag="ps")
            for m in range(8):
                nc.tensor.matmul(
                    ps_at[:, m:m + 1],
                    t_enc[:, m * 128:(m + 1) * 128],
                    wn[:],
                    start=True,
                    stop=True,
                )
            nc.vector.tensor_copy(cat[:, 8:16], ps_at[:])

            # ---------- combine + relu (sharded over output dim) -----------
            ps_x = psp.tile([1, 128], f32, tag="ps")
            for k in range(17):
                nc.tensor.matmul(
                    ps_x[:],
                    cat[:, k:k + 1],
                    t_combwt[:, k * 128:(k + 1) * 128],
                    start=(k == 0),
                    stop=(k == 16),
                )
            x_sb = sb.tile([1, 128], f32, tag="x_sb")
            nc.scalar.activation(x_sb[:], ps_x[:], AF.Relu)

            d_xin = dram.tile([128], f32, tag="d_xin")
            d_xag = dram.tile([H], f32, tag="d_xag")
            nc.sync.dma_start(d_xin[:], x_sb[:])
            nc.gpsimd.collective_compute(
                "AllGather",
                mybir.AluOpType.bypass,
                replica_groups=RG,
                ins=[d_xin[:].opt()],
                outs=[d_xag[:].opt()],
            )
            x_cols = sb.tile([128, 9], f32, tag="x_cols")
            nc.sync.dma_start(
                x_cols[:, 0:8], d_xag[:].rearrange("(p k) -> p k", k=8)
            )
            nc.vector.tensor_copy(x_cols[:, 8:9], t_consts[:, 0:1])

            # ---------- GRU (gates sharded: this core's 128 rows each) -----
            ps_grz = psp.tile([1, 256], f32, tag="ps")
            ps_gin = psp.tile([1, 128], f32, tag="ps")
            ps_ghn = psp.tile([1, 128], f32, tag="ps")
            # h-side first: runs while the x AllGather is still in flight
            for k in range(9):
                nc.tensor.matmul(
                    ps_ghn[:], t_hcols[:, k:k + 1],
                    t_hw[:, k * 384 + 256:(k + 1) * 384],
                    start=(k == 0), stop=(k == 8),
                )
            for k in range(9):
                nc.tensor.matmul(
                    ps_grz[:], t_hcols[:, k:k + 1],
                    t_hw[:, k * 384:k * 384 + 256],
                    start=(k == 0), stop=False,
                )
            for k in range(9):
                nc.tensor.matmul(
                    ps_grz[:], x_cols[:, k:k + 1],
                    t_xw[:, k * 384:k * 384 + 256],
                    start=False, stop=(k == 8),
                )
            for k in range(9):
                nc.tensor.matmul(
                    ps_gin[:], x_cols[:, k:k + 1],
                    t_xw[:, k * 384 + 256:(k + 1) * 384],
                    start=(k == 0), stop=(k == 8),
                )
            rz = sb.tile([1, 256], f32, tag="rz")
            nc.scalar.activation(rz[:], ps_grz[:], AF.Sigmoid)
            t2 = sb.tile([1, 128], f32, tag="t2")
            nc.vector.tensor_mul(t2[:], rz[:, 0:128], ps_ghn[:])
            t3 = sb.tile([1, 128], f32, tag="t3")
            nc.vector.tensor_add(t3[:], ps_gin[:], t2[:])
            n_t = sb.tile([1, 128], f32, tag="n_t")
            nc.scalar.activation(n_t[:], t3[:], AF.Tanh)
            t4 = sb.tile([1, 128], f32, tag="t4")
            nc.vector.tensor_sub(t4[:], t_hchunk[:], n_t[:])
            t5 = sb.tile([1, 128], f32, tag="t5")
            nc.vector.tensor_mul(t5[:], rz[:, 128:256], t4[:])
            hn = sb.tile([1, 128], f32, tag="hn")
            nc.vector.tensor_add(hn[:], n_t[:], t5[:])

            d_hin = dram.tile([128], f32, tag="d_hin")
            d_hag = dram.tile([H], f32, tag="d_hag")
            nc.sync.dma_start(d_hin[:], hn[:])
            nc.gpsimd.collective_compute(
                "AllGather",
                mybir.AluOpType.bypass,
                replica_groups=RG,
                ins=[d_hin[:].opt()],
                outs=[d_hag[:].opt()],
            )
            nc.sync.dma_start(o_h[:], d_hag[:])
            h2f = sb.tile([128, 8], f32, tag="h2f")
            nc.sync.dma_start(h2f[:], d_hag[:].rearrange("(p k) -> p k", k=8))
            h2 = sb.tile([128, 8], bf16, tag="h2")
            nc.vector.tensor_copy(h2[:], h2f[:])

            # ---------- big vocab matmul (bf16 weights, streamed) ----------
            # big-tile DMAs ride the scalar-engine HWDGE ring so they do not
            # delay collective triggers (gpsimd) or small chain DMAs (sync).
            d_logits = dram.tile([VS], f32, tag="d_logits")
            s_parts = sb.tile([1, NSL], f32, tag="s_parts")
            esc = sb.tile([1, NSLICE], f32, tag="esc")
            for tg in range(NTG):
                tiles = []
                for k in range(8):
                    tw = bigp.tile([128, TGC], bf16, tag="bw")
                    nc.scalar.dma_start(
                        tw[:], p_outwt[k, :, tg * TGC:(tg + 1) * TGC]
                    )
                    tiles.append(tw)
                for n in range(NSPT):
                    i = tg * NSPT + n
                    off = i * NSLICE
                    ps_b = psbig.tile([1, NSLICE], f32, tag="bps")
                    for k in range(8):
                        nc.tensor.matmul(
                            ps_b[:],
                            h2[:, k:k + 1],
                            tiles[k][:, n * NSLICE:(n + 1) * NSLICE],
                            start=(k == 0),
                            stop=(k == 7),
                        )
                    sl = slp.tile([1, NSLICE], f32, tag="sl")
                    nc.vector.tensor_add(sl[:], ps_b[:], t_outb[:, off:off + NSLICE])
                    nc.sync.dma_start(d_logits[off:off + NSLICE], sl[:])
                    # local sum(exp) accumulated per slice, off the critical path
                    nc.scalar.activation(
                        esc[:], sl[:], AF.Exp, accum_out=s_parts[:, i:i + 1]
                    )

            # ---------- epilogue: global log-sum-exp + subtract ------------
            s_loc = sb.tile([1, 1], f32, tag="s_loc")
            nc.vector.reduce_sum(s_loc[:], s_parts[:], axis=mybir.AxisListType.X)
            s_st = sb.tile([1, 8], f32, tag="s_st")
            nc.vector.memset(s_st[:], 0.0)
            nc.vector.tensor_copy(s_st[:, 0:1], s_loc[:])
            d_sin = dram.tile([8], f32, tag="d_sin")
            d_sag = dram.tile([8 * NCORES], f32, tag="d_sag")
            nc.sync.dma_start(d_sin[:], s_st[:])
            nc.gpsimd.collective_compute(
                "AllGather",
                mybir.AluOpType.bypass,
                replica_groups=RG,
                ins=[d_sin[:].opt()],
                outs=[d_sag[:].opt()],
            )
            # relayout DMA only needs d_logits — overlaps the s AllGather
            lsq = sb.tile([ROWS, JW], f32, tag="lsq")
            nc.sync.dma_start(lsq[:], d_logits[:].rearrange("(p j) -> p j", j=JW))
            s_row = sb.tile([1, 8 * NCORES], f32, tag="s_row")
            nc.sync.dma_start(s_row[:], d_sag[:])
            stot = sb.tile([1, 1], f32, tag="stot")
            nc.vector.reduce_sum(stot[:], s_row[:], axis=mybir.AxisListType.X)
            lse = sb.tile([1, 1], f32, tag="lse")
            nc.scalar.activation(lse[:], stot[:], AF.Ln)
            ps_lb = psp.tile([128, 1], f32, tag="ps")
            nc.tensor.matmul(ps_lb[:], t_ones[:], lse[:], start=True, stop=True)
            lseb = sb.tile([128, 1], f32, tag="lseb")
            nc.vector.tensor_copy(lseb[:], ps_lb[:])
            fin = sb.tile([ROWS, JW], f32, tag="fin")
            nc.vector.tensor_scalar(
                fin[:], lsq[:], lseb[:], None,
                op0=mybir.AluOpType.subtract,
            )
            nc.sync.dma_start(
                o_logits[:].rearrange("(p j) -> p j", j=JW), fin[:]
            )

    return nc


def _host_prep(inputs):
    token = int(np.asarray(inputs["token"]).reshape(-1)[0])
    emb = np.asarray(inputs["emb"], dtype=np.float32)
    embedded = emb[token].astype(np.float32)                    # [H]
    h = np.asarray(inputs["hidden"], dtype=np.float32).reshape(H)
    enc = np.ascontiguousarray(np.asarray(inputs["encoder_outputs"], np.float32))
    attn_W = np.asarray(inputs["attn_W"], np.float32)           # [L, 2H]
    attn_b = np.asarray(inputs["attn_b"], np.float32)           # [L]
    comb_W = np.asarray(inputs["comb_W"], np.float32)           # [H, 2H]
    comb_b = np.asarray(inputs["comb_b"], np.float32)           # [H]
    W_ih = np.asarray(inputs["W_ih"], np.float32)               # [3H, H]
    W_hh = np.asarray(inputs["W_hh"], np.float32)
    b_ih = np.asarray(inputs["b_ih"], np.float32)
    b_hh = np.asarray(inputs["b_hh"], np.float32)
    out_W = np.asarray(inputs["out_W"], np.float32)             # [V, H]
    out_b = np.asarray(inputs["out_b"], np.float32)             # [V]

    eh = np.concatenate([embedded, h])                          # [2H]
    eh_cols = np.zeros((128, 17), np.float32)
    eh_cols[:, :16] = eh.reshape(16, 128).T
    eh_cols[0, 16] = 1.0

    attnwt = np.zeros((128, 17 * L), np.float32)
    attnwt[:, :16 * L] = (
        attn_W.T.reshape(16, 128, L).transpose(1, 0, 2).reshape(128, 16 * L)
    )
    attnwt[0, 16 * L:] = attn_b

    emb_cols = np.ascontiguousarray(embedded.reshape(8, 128).T)

    h_cols = np.zeros((128, 9), np.float32)
    h_cols[:, :8] = h.reshape(128, 8)
    h_cols[0, 8] = 1.0

    consts = np.zeros((128, 2), np.float32)
    consts[0, 0] = 1.0
    consts[:, 1] = 1.0
    ones_row = np.ones((1, 128), np.float32)

    common = {
        "eh_cols": eh_cols, "attnwt": attnwt, "enc": enc,
        "emb_cols": emb_cols, "h_cols": h_cols,
        "consts": consts, "ones_row": ones_row,
    }

    in_maps = []
    for c in range(NCORES):
        m = dict(common)
        blk = comb_W[c * 128:(c + 1) * 128, :]                  # [128, 2H]
        combwt = np.zeros((128, 17 * 128), np.float32)
        combwt[:, :16 * 128] = (
            blk.T.reshape(16, 128, 128).transpose(1, 0, 2).reshape(128, 16 * 128)
        )
        combwt[0, 16 * 128:] = comb_b[c * 128:(c + 1) * 128]
        m["combwt"] = combwt

        gidx = np.concatenate([
            np.arange(128) + c * 128,
            np.arange(128) + H + c * 128,
            np.arange(128) + 2 * H + c * 128,
        ])
        for nm, W, b in (("xw", W_ih, b_ih), ("hw", W_hh, b_hh)):
            Wg = W[gidx, :]                                     # [384, H]
            t = np.zeros((128, 9 * 384), np.float32)
            t[:, :8 * 384] = Wg.T.reshape(128, 8, 384).reshape(128, 8 * 384)
            t[0, 8 * 384:] = b[gidx]
            m[nm] = t

        m["h_chunk"] = h[c * 128:(c + 1) * 128][None, :].copy()

        shard = out_W[c * VS:(c + 1) * VS, :]                   # [VS, H]
        import ml_dtypes
        m["outwt"] = np.ascontiguousarray(
            shard.T.reshape(128, 8, VS).transpose(1, 0, 2)
        ).astype(ml_dtypes.bfloat16)
        m["outb_p0"] = np.ascontiguousarray(
            out_b[c * VS:(c + 1) * VS][None, :]
        )
        in_maps.append(m)
    return in_maps


def kernel(**inputs):
    global LAST_EXEC_NS, LAST_RESULTS
    from concourse.bass_utils import run_bass_kernel_spmd

    if "nc" not in _BUILD_CACHE:
        nc = _build_nc()
        if not nc.is_finalized():
            nc.finalize()
        _BUILD_CACHE["nc"] = nc
    nc = _BUILD_CACHE["nc"]

    in_maps = _host_prep(inputs)
    trace = os.environ.get("KERNEL_PROFILE", "0") == "1"
    kw = {}
    if trace:
        try:
            _install_profile_shims()
            kw["tmpdir"] = os.environ.get("KERNEL_TRACE_DIR") or None
        except Exception as e:  # profiling is best-effort
            print(f"profile shim failed: {e}", file=sys.stderr)
            trace = False
    res = run_bass_kernel_spmd(
        nc, in_maps, core_ids=list(range(NCORES)), trace=trace, **kw,
    )
    LAST_EXEC_NS = getattr(res, "exec_time_ns", None)
    LAST_RESULTS = res

    outs = res.results
    logits = np.concatenate(
        [np.asarray(outs[c]["out_logits"], np.float32) for c in range(NCORES)]
    )[None, :]
    h_new = np.asarray(outs[0]["out_h"], np.float32)[None, None, :]
    attnw = np.asarray(outs[0]["out_attnw"], np.float32)[None, :]
    return logits, h_new, attnw
